# revision 22
# baseline (speedup 1.0000x reference)
"""Fused cross-entropy loss over a 100k item vocabulary on 8 Trainium2 cores.

Math (matches the reference):
    logits = hidden_flat @ item_emb.T          # [1024, 100000]
    nll[r] = log(sum_v exp(logits[r, v])) - logits[r, label[r]]
    loss   = sum(w * nll) / sum(w)             # w = active-token mask

Sharding: the vocab dim is split across the 8 cores (12500 each). Every core
computes partial row-sums S_c[r] = sum_{v in shard} exp(logits[r, v]) with
bf16 matmuls (fp32 PSUM accumulate) + a fused ACT exp/row-sum, then one tiny
AllReduce (4 KB) combines the denominators. Label logits are computed exactly
in fp32 (row-wise dot products) redundantly on every core, so no second
collective is needed. The final masked mean is computed on-device.
"""
import sys

try:
    import concourse.bass as _cb  # provided by the environment boot path
except ModuleNotFoundError:
    sys.path.insert(0, "/opt/trn_rl_repo")

import numpy as np

import concourse.bass as bass
import concourse.bacc as bacc
import concourse.tile as tile
import concourse.mybir as mybir
from concourse import bass_utils

N_CORES = 8
B, L, D = 8, 128, 768
V = 100000
VS = V // N_CORES            # vocab shard per core
T = B * L                    # 1024 token rows (last row per batch is masked out)
KC = D // 128                # contraction chunks
NUM_USERS = 10000
LABEL_OFFSET = 151669 + NUM_USERS

BF16 = mybir.dt.bfloat16
F32 = mybir.dt.float32
FP8 = mybir.dt.float8e4
NP_BF16 = mybir.dt.np(BF16)
NP_FP8 = mybir.dt.np(FP8)

USE_FP8 = True
EMB_SCALE = 32.0  # emb pre-scaled into fp8's sweet spot; undone via ACT scale
KC2 = D // 256  # DoubleRow contraction chunks

# vocab chunks per core (psum tile = 4 banks = 2048 fp32). Small chunks first
# so the exp pipeline starts as soon as the first slice of emb lands.
CHUNK_W = 2048
_widths = [512] + [2048] * 5 + [VS - 512 - 5 * 2048]
assert sum(_widths) == VS and all(0 < w <= CHUNK_W for w in _widths)
CHUNKS = []
_off = 0
for _w in _widths:
    CHUNKS.append((_off, _w))
    _off += _w

_prog_cache = {}


def build_program(repeat: int = 1, sim_single_core: bool = False):
    key = (repeat, sim_single_core)
    if key in _prog_cache:
        return _prog_cache[key]
    nc = bacc.Bacc(
        "TRN2",
        target_bir_lowering=False,
        debug=False,
        enable_asserts=True,
        num_devices=1 if sim_single_core else N_CORES,
    )
    if USE_FP8:
        hT = nc.dram_tensor("hT", [128, KC2, 2, T], FP8, kind="ExternalInput")
        eT = nc.dram_tensor("eT", [128, KC2, 2, VS], FP8, kind="ExternalInput")
    else:
        hT = nc.dram_tensor("hT", [D, T], BF16, kind="ExternalInput")
        eT = nc.dram_tensor("eT", [D, VS], BF16, kind="ExternalInput")
    hpb = nc.dram_tensor("hpb", [128, B * D], F32, kind="ExternalInput")
    gpb = nc.dram_tensor("gpb", [128, B * D], F32, kind="ExternalInput")
    wpb = nc.dram_tensor("wpb", [128, B], F32, kind="ExternalInput")
    loss = nc.dram_tensor("loss", [1, 1], F32, kind="ExternalOutput")

    add = mybir.AluOpType.add
    mult = mybir.AluOpType.mult
    AF = mybir.ActivationFunctionType
    AX = mybir.AxisListType

    with tile.TileContext(nc) as tc:
        with (
            tc.tile_pool(name="const", bufs=1) as cpool,
            tc.tile_pool(name="rhs", bufs=3) as rpool,
            tc.tile_pool(name="psum", bufs=2, space="PSUM") as ppool,
            tc.tile_pool(name="dram", bufs=1, space="DRAM") as dpool,
        ):
            # resident tensors
            if USE_FP8:
                ht_sb = cpool.tile([128, KC2, 2, T], FP8)
                # block-0 weights first so the first matmul isn't gated on
                # the full hidden transfer
                nc.sync.dma_start(ht_sb[:, :, :, 0:128], hT.ap()[:, :, :, 0:128])
                nc.sync.dma_start(ht_sb[:, :, :, 128:T], hT.ap()[:, :, :, 128:T])
            else:
                ht_sb = cpool.tile([128, KC, T], BF16)
                nc.sync.dma_start(
                    ht_sb[:], hT.ap().rearrange("(k p) t -> p k t", p=128)
                )
            # main loop: partial exp row-sums over this core's vocab shard
            r_sb = cpool.tile([128, B, len(CHUNKS)], F32)
            if not USE_FP8:
                eT_r = eT.ap().rearrange("(k p) v -> p k v", p=128)

            def main_loop(_iv=None):
                for ci, (jstart, W) in enumerate(CHUNKS):
                    nbank = (W + 511) // 512
                    if USE_FP8:
                        rt = rpool.tile(
                            [128, KC2, 2, CHUNK_W], FP8, tag="rt", name=f"rt{ci}"
                        )
                        nc.sync.dma_start(
                            rt[:, :, :, :W], eT.ap()[:, :, :, jstart : jstart + W]
                        )
                    else:
                        rt = rpool.tile(
                            [128, KC, CHUNK_W], BF16, tag="rt", name=f"rt{ci}"
                        )
                        nc.sync.dma_start(
                            rt[:, :, :W], eT_r[:, :, jstart : jstart + W]
                        )
                    for i in range(B):
                        pt = ppool.tile([128, CHUNK_W], F32, tag="pt", name=f"pt{ci}_{i}")
                        if USE_FP8:
                            for k in range(KC2):
                                for b in range(nbank):
                                    s = 512 * b
                                    e = min(W, s + 512)
                                    nc.tensor.matmul(
                                        pt[:, s:e],
                                        lhsT=ht_sb[:, k, :, i * 128 : (i + 1) * 128],
                                        rhs=rt[:, k, :, s:e],
                                        perf_mode=mybir.MatmulPerfMode.DoubleRow,
                                        start=(k == 0),
                                        stop=(k == KC2 - 1),
                                    )
                        else:
                            for k in range(KC):
                                for b in range(nbank):
                                    s = 512 * b
                                    e = min(W, s + 512)
                                    nc.tensor.matmul(
                                        pt[:, s:e],
                                        lhsT=ht_sb[:, k, i * 128 : (i + 1) * 128],
                                        rhs=rt[:, k, s:e],
                                        start=(k == 0),
                                        stop=(k == KC - 1),
                                    )
                        # exp in place in PSUM; only the accumulated row-sum
                        # is consumed downstream
                        nc.scalar.activation(
                            pt[:, :W],
                            pt[:, :W],
                            AF.Exp,
                            scale=(1.0 / EMB_SCALE) if USE_FP8 else 1.0,
                            accum_out=r_sb[:, i, ci : ci + 1],
                        )

            if repeat == 1:
                main_loop()
            else:
                with tc.For_i(0, repeat, 1) as iv:
                    main_loop(iv)

            # constants + exact fp32 label logits (DVE/DMA work overlapping
            # the PE/ACT main loop; results only needed in the epilogue)
            hpb_sb = cpool.tile([128, B * D], F32)
            nc.sync.dma_start(hpb_sb[:], hpb.ap())
            gpb_sb = cpool.tile([128, B * D], F32)
            nc.sync.dma_start(gpb_sb[:], gpb.ap())
            wpb_sb = cpool.tile([128, B], F32)
            nc.sync.dma_start(wpb_sb[:], wpb.ap())
            ones_sb = cpool.tile([128, 1], F32)
            nc.vector.memset(ones_sb[:], 1.0)

            dot_sb = cpool.tile([128, B], F32)
            tscr = cpool.tile([128, D], F32)
            for i in range(B):
                nc.vector.tensor_mul(
                    tscr[:],
                    hpb_sb[:, i * D : (i + 1) * D],
                    gpb_sb[:, i * D : (i + 1) * D],
                )
                nc.vector.tensor_reduce(
                    out=dot_sb[:, i : i + 1], in_=tscr[:], axis=AX.X, op=add
                )

            n2 = cpool.tile([128, 2], F32)
            nc.vector.tensor_reduce(
                out=n2[:, 1:2], in_=wpb_sb[:], axis=AX.X, op=add
            )

            s_sb = cpool.tile([128, B], F32)
            nc.vector.tensor_reduce(out=s_sb[:], in_=r_sb[:], axis=AX.X, op=add)

            if sim_single_core:
                stot = s_sb
            else:
                # AllGather the partial softmax denominators (4 KB per core;
                # cheaper floor than AllReduce) and sum the 8 shards locally.
                cc_in = dpool.tile([128, B], F32)
                cc_out = dpool.tile([N_CORES, 128, B], F32, addr_space="Shared")
                nc.sync.dma_start(cc_in[:], s_sb[:])
                nc.gpsimd.collective_compute(
                    "AllGather",
                    mybir.AluOpType.bypass,
                    replica_groups=[list(range(N_CORES))],
                    ins=[cc_in.opt()],
                    outs=[cc_out.opt()],
                )
                sall = cpool.tile([128, N_CORES, B], F32)
                nc.sync.dma_start(
                    sall[:], cc_out.rearrange("r p i -> p r i")
                )
                stot = cpool.tile([128, B], F32)
                nc.vector.tensor_add(stot[:], sall[:, 0, :], sall[:, 1, :])
                for r in range(2, N_CORES):
                    nc.vector.tensor_add(stot[:], stot[:], sall[:, r, :])

            # loss = sum(w * (ln(S) - dot)) / sum(w)
            lt = cpool.tile([128, B], F32)
            nc.scalar.activation(lt[:], stot[:], AF.Ln)
            u = cpool.tile([128, B], F32)
            nc.vector.tensor_sub(u[:], lt[:], dot_sb[:])
            nc.vector.tensor_mul(u[:], u[:], wpb_sb[:])
            nc.vector.tensor_reduce(out=n2[:, 0:1], in_=u[:], axis=AX.X, op=add)
            ps2 = ppool.tile([1, 2], F32, tag="pt", name="ps2")
            nc.tensor.matmul(ps2[:], lhsT=ones_sb[:], rhs=n2[:], start=True, stop=True)
            inv = cpool.tile([1, 1], F32)
            nc.vector.reciprocal(inv[:], ps2[:, 1:2])
            res = cpool.tile([1, 1], F32)
            nc.vector.tensor_mul(res[:], ps2[:, 0:1], inv[:])
            nc.sync.dma_start(loss.ap(), res[:])

    nc.compile()
    _prog_cache[repeat] = nc
    return nc


def prepare_in_maps(hidden, item_emb, labels_main, attention_mask, prompt_length):
    hidden = np.asarray(hidden, dtype=np.float32).reshape(B, L, D)
    item_emb = np.asarray(item_emb, dtype=np.float32).reshape(V, D)
    labels_main = np.asarray(labels_main).reshape(B, L)
    attention_mask = np.asarray(attention_mask)
    pl = int(prompt_length)

    active = attention_mask[:, pl + 1 :] == 1  # [B, L-1]
    assert active.shape == (B, L - 1), active.shape

    hidden_T = hidden.reshape(T, D).T  # [D, T] f32
    if USE_FP8:
        # d = k*256 + two*128 + p  ->  [p, k, two, t]
        hT = np.ascontiguousarray(
            hidden_T.reshape(KC2, 2, 128, T).transpose(2, 0, 1, 3).astype(NP_FP8)
        )
    else:
        hT = np.ascontiguousarray(hidden_T.astype(NP_BF16))  # [D, T] bf16
    hpb = np.ascontiguousarray(
        hidden.transpose(1, 0, 2).reshape(128, B * D)
    )  # [p, i*D+d]

    lab = np.zeros((128, B), dtype=np.int64)
    lab[: L - 1, :] = np.clip(
        labels_main[:, 1:].T - LABEL_OFFSET, 0, V - 1
    )
    gpb = np.ascontiguousarray(
        item_emb[lab.reshape(-1)].reshape(128, B * D)
    )

    w = np.zeros((128, B), dtype=np.float32)
    w[: L - 1, :] = active.T.astype(np.float32)

    if USE_FP8:
        emb_T = (item_emb.T * EMB_SCALE).astype(NP_FP8)  # [D, V]
        eT = np.ascontiguousarray(
            emb_T.reshape(KC2, 2, 128, V).transpose(2, 0, 1, 3)
        )  # [128, KC2, 2, V]
        shards = [
            np.ascontiguousarray(eT[:, :, :, c * VS : (c + 1) * VS])
            for c in range(N_CORES)
        ]
    else:
        eT = np.ascontiguousarray(item_emb.astype(NP_BF16).T)  # [D, V] bf16
        shards = [
            np.ascontiguousarray(eT[:, c * VS : (c + 1) * VS])
            for c in range(N_CORES)
        ]

    in_maps = []
    for c in range(N_CORES):
        in_maps.append(
            {
                "hT": hT,
                "eT": shards[c],
                "hpb": hpb,
                "gpb": gpb,
                "wpb": w,
            }
        )
    return in_maps


def kernel(hidden, item_emb, labels_main, attention_mask, prompt_length):
    in_maps = prepare_in_maps(
        hidden, item_emb, labels_main, attention_mask, prompt_length
    )
    nc = build_program()
    res = bass_utils.run_bass_kernel_spmd(
        nc, in_maps, core_ids=list(range(N_CORES))
    )
    return np.float32(res.results[0]["loss"][0, 0])


# revision 24
# speedup vs baseline: 1.0034x; 1.0034x over previous
"""Fused cross-entropy loss over a 100k item vocabulary on 8 Trainium2 cores.

Math (matches the reference):
    logits = hidden_flat @ item_emb.T          # [1024, 100000]
    nll[r] = log(sum_v exp(logits[r, v])) - logits[r, label[r]]
    loss   = sum(w * nll) / sum(w)             # w = active-token mask

Sharding: the vocab dim is split across the 8 cores (12500 each). Every core
computes partial row-sums S_c[r] = sum_{v in shard} exp(logits[r, v]) with
bf16 matmuls (fp32 PSUM accumulate) + a fused ACT exp/row-sum, then one tiny
AllReduce (4 KB) combines the denominators. Label logits are computed exactly
in fp32 (row-wise dot products) redundantly on every core, so no second
collective is needed. The final masked mean is computed on-device.
"""
import sys

try:
    import concourse.bass as _cb  # provided by the environment boot path
except ModuleNotFoundError:
    sys.path.insert(0, "/opt/trn_rl_repo")

import numpy as np

import concourse.bass as bass
import concourse.bacc as bacc
import concourse.tile as tile
import concourse.mybir as mybir
from concourse import bass_utils

N_CORES = 8
B, L, D = 8, 128, 768
V = 100000
VS = V // N_CORES            # vocab shard per core
T = B * L                    # 1024 token rows (last row per batch is masked out)
KC = D // 128                # contraction chunks
NUM_USERS = 10000
LABEL_OFFSET = 151669 + NUM_USERS

BF16 = mybir.dt.bfloat16
F32 = mybir.dt.float32
FP8 = mybir.dt.float8e4
NP_BF16 = mybir.dt.np(BF16)
NP_FP8 = mybir.dt.np(FP8)

USE_FP8 = True
EMB_SCALE = 32.0  # emb pre-scaled into fp8's sweet spot; undone via ACT scale
KC2 = D // 256  # DoubleRow contraction chunks

# vocab chunks per core (psum tile = 4 banks = 2048 fp32). Small chunks first
# so the exp pipeline starts as soon as the first slice of emb lands.
CHUNK_W = 2048
_widths = [512] + [2048] * 5 + [VS - 512 - 5 * 2048]
assert sum(_widths) == VS and all(0 < w <= CHUNK_W for w in _widths)
CHUNKS = []
_off = 0
for _w in _widths:
    CHUNKS.append((_off, _w))
    _off += _w

_prog_cache = {}


def build_program(repeat: int = 1, sim_single_core: bool = False):
    key = (repeat, sim_single_core)
    if key in _prog_cache:
        return _prog_cache[key]
    nc = bacc.Bacc(
        "TRN2",
        target_bir_lowering=False,
        debug=False,
        enable_asserts=True,
        num_devices=1 if sim_single_core else N_CORES,
    )
    if USE_FP8:
        hT = nc.dram_tensor("hT", [128, KC2, 2, T], FP8, kind="ExternalInput")
        eT = nc.dram_tensor("eT", [128, KC2, 2, VS], FP8, kind="ExternalInput")
    else:
        hT = nc.dram_tensor("hT", [D, T], BF16, kind="ExternalInput")
        eT = nc.dram_tensor("eT", [D, VS], BF16, kind="ExternalInput")
    hpb = nc.dram_tensor("hpb", [128, B * D], F32, kind="ExternalInput")
    gpb = nc.dram_tensor("gpb", [128, B * D], F32, kind="ExternalInput")
    wpb = nc.dram_tensor("wpb", [128, B], F32, kind="ExternalInput")
    loss = nc.dram_tensor("loss", [1, 1], F32, kind="ExternalOutput")

    add = mybir.AluOpType.add
    mult = mybir.AluOpType.mult
    AF = mybir.ActivationFunctionType
    AX = mybir.AxisListType

    with tile.TileContext(nc) as tc:
        with (
            tc.tile_pool(name="const", bufs=1) as cpool,
            tc.tile_pool(name="rhs", bufs=3) as rpool,
            tc.tile_pool(name="psum", bufs=2, space="PSUM") as ppool,
            tc.tile_pool(name="dram", bufs=1, space="DRAM") as dpool,
        ):
            # resident tensors
            if USE_FP8:
                # first vocab chunk + t-block-0 weights land before the bulk
                # hidden transfer so the pipeline starts immediately
                rt0 = rpool.tile([128, KC2, 2, CHUNK_W], FP8, tag="rt", name="rt0")
                W0 = CHUNKS[0][1]
                nc.sync.dma_start(rt0[:, :, :, :W0], eT.ap()[:, :, :, 0:W0])
                ht_sb = cpool.tile([128, KC2, 2, T], FP8)
                nc.sync.dma_start(ht_sb[:, :, :, 0:128], hT.ap()[:, :, :, 0:128])
                nc.sync.dma_start(ht_sb[:, :, :, 128:T], hT.ap()[:, :, :, 128:T])
            else:
                ht_sb = cpool.tile([128, KC, T], BF16)
                nc.sync.dma_start(
                    ht_sb[:], hT.ap().rearrange("(k p) t -> p k t", p=128)
                )
            # main loop: partial exp row-sums over this core's vocab shard
            r_sb = cpool.tile([128, B, len(CHUNKS)], F32)
            if not USE_FP8:
                eT_r = eT.ap().rearrange("(k p) v -> p k v", p=128)

            def main_loop(_iv=None):
                for ci, (jstart, W) in enumerate(CHUNKS):
                    nbank = (W + 511) // 512
                    if USE_FP8:
                        if ci == 0:
                            rt = rt0
                        else:
                            rt = rpool.tile(
                                [128, KC2, 2, CHUNK_W], FP8, tag="rt", name=f"rt{ci}"
                            )
                            nc.sync.dma_start(
                                rt[:, :, :, :W], eT.ap()[:, :, :, jstart : jstart + W]
                            )
                    else:
                        rt = rpool.tile(
                            [128, KC, CHUNK_W], BF16, tag="rt", name=f"rt{ci}"
                        )
                        nc.sync.dma_start(
                            rt[:, :, :W], eT_r[:, :, jstart : jstart + W]
                        )
                    for i in range(B):
                        pt = ppool.tile([128, CHUNK_W], F32, tag="pt", name=f"pt{ci}_{i}")
                        if USE_FP8:
                            for k in range(KC2):
                                for b in range(nbank):
                                    s = 512 * b
                                    e = min(W, s + 512)
                                    nc.tensor.matmul(
                                        pt[:, s:e],
                                        lhsT=ht_sb[:, k, :, i * 128 : (i + 1) * 128],
                                        rhs=rt[:, k, :, s:e],
                                        perf_mode=mybir.MatmulPerfMode.DoubleRow,
                                        start=(k == 0),
                                        stop=(k == KC2 - 1),
                                    )
                        else:
                            for k in range(KC):
                                for b in range(nbank):
                                    s = 512 * b
                                    e = min(W, s + 512)
                                    nc.tensor.matmul(
                                        pt[:, s:e],
                                        lhsT=ht_sb[:, k, i * 128 : (i + 1) * 128],
                                        rhs=rt[:, k, s:e],
                                        start=(k == 0),
                                        stop=(k == KC - 1),
                                    )
                        # exp in place in PSUM; only the accumulated row-sum
                        # is consumed downstream
                        nc.scalar.activation(
                            pt[:, :W],
                            pt[:, :W],
                            AF.Exp,
                            scale=(1.0 / EMB_SCALE) if USE_FP8 else 1.0,
                            accum_out=r_sb[:, i, ci : ci + 1],
                        )

            if repeat == 1:
                main_loop()
            else:
                with tc.For_i(0, repeat, 1) as iv:
                    main_loop(iv)

            # constants + exact fp32 label logits (DVE/DMA work overlapping
            # the PE/ACT main loop; results only needed in the epilogue)
            hpb_sb = cpool.tile([128, B * D], F32)
            nc.sync.dma_start(hpb_sb[:], hpb.ap())
            gpb_sb = cpool.tile([128, B * D], F32)
            nc.sync.dma_start(gpb_sb[:], gpb.ap())
            wpb_sb = cpool.tile([128, B], F32)
            nc.sync.dma_start(wpb_sb[:], wpb.ap())
            ones_sb = cpool.tile([128, 1], F32)
            nc.vector.memset(ones_sb[:], 1.0)

            dot_sb = cpool.tile([128, B], F32)
            tscr = cpool.tile([128, D], F32)
            for i in range(B):
                nc.vector.tensor_mul(
                    tscr[:],
                    hpb_sb[:, i * D : (i + 1) * D],
                    gpb_sb[:, i * D : (i + 1) * D],
                )
                nc.vector.tensor_reduce(
                    out=dot_sb[:, i : i + 1], in_=tscr[:], axis=AX.X, op=add
                )

            n2 = cpool.tile([128, 2], F32)
            nc.vector.tensor_reduce(
                out=n2[:, 1:2], in_=wpb_sb[:], axis=AX.X, op=add
            )

            s_sb = cpool.tile([128, B], F32)
            nc.vector.tensor_reduce(out=s_sb[:], in_=r_sb[:], axis=AX.X, op=add)

            if sim_single_core:
                stot = s_sb
            else:
                # AllGather the partial softmax denominators (4 KB per core;
                # cheaper floor than AllReduce) and sum the 8 shards locally.
                cc_in = dpool.tile([128, B], F32)
                cc_out = dpool.tile([N_CORES, 128, B], F32, addr_space="Shared")
                nc.sync.dma_start(cc_in[:], s_sb[:])
                nc.gpsimd.collective_compute(
                    "AllGather",
                    mybir.AluOpType.bypass,
                    replica_groups=[list(range(N_CORES))],
                    ins=[cc_in.opt()],
                    outs=[cc_out.opt()],
                )
                sall = cpool.tile([128, N_CORES, B], F32)
                nc.sync.dma_start(
                    sall[:], cc_out.rearrange("r p i -> p r i")
                )
                stot = cpool.tile([128, B], F32)
                nc.vector.tensor_add(stot[:], sall[:, 0, :], sall[:, 1, :])
                for r in range(2, N_CORES):
                    nc.vector.tensor_add(stot[:], stot[:], sall[:, r, :])

            # loss = sum(w * (ln(S) - dot)) / sum(w)
            lt = cpool.tile([128, B], F32)
            nc.scalar.activation(lt[:], stot[:], AF.Ln)
            u = cpool.tile([128, B], F32)
            nc.vector.tensor_sub(u[:], lt[:], dot_sb[:])
            nc.vector.tensor_mul(u[:], u[:], wpb_sb[:])
            nc.vector.tensor_reduce(out=n2[:, 0:1], in_=u[:], axis=AX.X, op=add)
            ps2 = ppool.tile([1, 2], F32, tag="pt", name="ps2")
            nc.tensor.matmul(ps2[:], lhsT=ones_sb[:], rhs=n2[:], start=True, stop=True)
            inv = cpool.tile([1, 1], F32)
            nc.vector.reciprocal(inv[:], ps2[:, 1:2])
            res = cpool.tile([1, 1], F32)
            nc.vector.tensor_mul(res[:], ps2[:, 0:1], inv[:])
            nc.sync.dma_start(loss.ap(), res[:])

    nc.compile()
    _prog_cache[repeat] = nc
    return nc


def prepare_in_maps(hidden, item_emb, labels_main, attention_mask, prompt_length):
    hidden = np.asarray(hidden, dtype=np.float32).reshape(B, L, D)
    item_emb = np.asarray(item_emb, dtype=np.float32).reshape(V, D)
    labels_main = np.asarray(labels_main).reshape(B, L)
    attention_mask = np.asarray(attention_mask)
    pl = int(prompt_length)

    active = attention_mask[:, pl + 1 :] == 1  # [B, L-1]
    assert active.shape == (B, L - 1), active.shape

    hidden_T = hidden.reshape(T, D).T  # [D, T] f32
    if USE_FP8:
        # d = k*256 + two*128 + p  ->  [p, k, two, t]
        hT = np.ascontiguousarray(
            hidden_T.reshape(KC2, 2, 128, T).transpose(2, 0, 1, 3).astype(NP_FP8)
        )
    else:
        hT = np.ascontiguousarray(hidden_T.astype(NP_BF16))  # [D, T] bf16
    hpb = np.ascontiguousarray(
        hidden.transpose(1, 0, 2).reshape(128, B * D)
    )  # [p, i*D+d]

    lab = np.zeros((128, B), dtype=np.int64)
    lab[: L - 1, :] = np.clip(
        labels_main[:, 1:].T - LABEL_OFFSET, 0, V - 1
    )
    gpb = np.ascontiguousarray(
        item_emb[lab.reshape(-1)].reshape(128, B * D)
    )

    w = np.zeros((128, B), dtype=np.float32)
    w[: L - 1, :] = active.T.astype(np.float32)

    if USE_FP8:
        emb_T = (item_emb.T * EMB_SCALE).astype(NP_FP8)  # [D, V]
        eT = np.ascontiguousarray(
            emb_T.reshape(KC2, 2, 128, V).transpose(2, 0, 1, 3)
        )  # [128, KC2, 2, V]
        shards = [
            np.ascontiguousarray(eT[:, :, :, c * VS : (c + 1) * VS])
            for c in range(N_CORES)
        ]
    else:
        eT = np.ascontiguousarray(item_emb.astype(NP_BF16).T)  # [D, V] bf16
        shards = [
            np.ascontiguousarray(eT[:, c * VS : (c + 1) * VS])
            for c in range(N_CORES)
        ]

    in_maps = []
    for c in range(N_CORES):
        in_maps.append(
            {
                "hT": hT,
                "eT": shards[c],
                "hpb": hpb,
                "gpb": gpb,
                "wpb": w,
            }
        )
    return in_maps


def kernel(hidden, item_emb, labels_main, attention_mask, prompt_length):
    in_maps = prepare_in_maps(
        hidden, item_emb, labels_main, attention_mask, prompt_length
    )
    nc = build_program()
    res = bass_utils.run_bass_kernel_spmd(
        nc, in_maps, core_ids=list(range(N_CORES))
    )
    return np.float32(res.results[0]["loss"][0, 0])
